# revision 21
# baseline (speedup 1.0000x reference)
"""Dilated (LongNet-style) attention kernel for 8 TRN2 NeuronCores.

Head-sharded SPMD design (core c owns heads {c, 8+c}), bf16 datapath:
  - Inputs stream in as bf16 (halves HBM traffic); all matmuls run bf16
    (enables fast-weight-load, which fp32r denies) with fp32 PSUM.
  - Per (branch, segment) job: scores are computed transposed ([key, query])
    in 512-col PSUM banks; the causal mask of the diagonal 128-block is
    pre-loaded into PSUM by a tiny identity x TRI matmul that also clears the
    bank (start=True), so no post-exp masking op is needed. exp runs on ACT
    into bf16 SBUF tiles; AV accumulates (with an appended ones column in V
    supplying the softmax denominators Z) into fp32 PSUM.
  - Branch merge = raw sum of exp-weighted AV and Z across branches
    (softmax-of-lse merge is algebraically A_tot/Z_tot), scattered into a
    dense fp32 accumulator; single normalization at the end.
  - Projection matmuls (and the first-half normalization) are emitted as
    filler quanta *between* the attention jc-steps so the PE never idles
    while ACT computes exp.
  - Tail: normalize -> two-chunk bf16 AllToAll (feature->position
    redistribution) overlapped with the per-chunk output projection.
"""

import sys

if "/opt/trn_rl_repo" not in sys.path:
    sys.path.insert(0, "/opt/trn_rl_repo")

import contextlib

import numpy as np
import ml_dtypes

import concourse.bacc as bacc
import concourse.bass as bass  # noqa: F401
import concourse.mybir as mybir
import concourse.tile as tile
from concourse import bass_utils

F32 = mybir.dt.float32
BF16 = mybir.dt.bfloat16
AF = mybir.ActivationFunctionType
BF = ml_dtypes.bfloat16

N_CORES = 8
E, L, H, D = 1024, 4096, 16, 64
KC = 8          # contraction chunks of 128 for the projections
PBP = 1024      # projection position block
NPP = L // PBP  # 4
PB = 512        # outproj position block (per core)
CW = 256        # a2a/outproj chunk width within each 512 block
G = 1024        # compressed segment length (all branches)
VBW = 65        # V_both per-chunk width (64 feats + ones col)

# constf columns: 0:2 bqk | 2:10 bo8 | 10:12 ws indicators
# constb columns: 0:128 tri | 128:256 eye | 256:272 ones


def _build():
    nc = bacc.Bacc("TRN2", target_bir_lowering=False, debug=False,
                   num_devices=N_CORES)

    qT = nc.dram_tensor("qT", [E, L], BF16, kind="ExternalInput")
    kT = nc.dram_tensor("kT", [E, L], BF16, kind="ExternalInput")
    vT = nc.dram_tensor("vT", [E, L], BF16, kind="ExternalInput")
    w3 = nc.dram_tensor("w3", [128, 3 * KC * 128], BF16, kind="ExternalInput")
    wo = nc.dram_tensor("wo", [128, 8 * E], BF16, kind="ExternalInput")
    constf = nc.dram_tensor("constf", [128, 12], F32, kind="ExternalInput")
    constb = nc.dram_tensor("constb", [128, 272], BF16, kind="ExternalInput")
    ind2d = nc.dram_tensor("ind2d", [2, 128], BF16, kind="ExternalInput")

    outT = nc.dram_tensor("outT", [E, PB], BF16, kind="ExternalOutput")

    a2a_warm_in = nc.dram_tensor("a2a_warm_in", [8, 1, 64], BF16)
    a2a_warm_out = nc.dram_tensor("a2a_warm_out", [8, 1, 64], BF16)
    a2a_in = [nc.dram_tensor(f"a2a_in{h}", [8, 128, CW], BF16)
              for h in range(2)]
    a2a_out = [nc.dram_tensor(f"a2a_out{h}", [8, 128, CW], BF16)
               for h in range(2)]

    def _emit(tc, ctx):
        pin = ctx.enter_context(tc.tile_pool(name="pin", bufs=4))
        persist = ctx.enter_context(tc.tile_pool(name="persist", bufs=1))
        vpool = ctx.enter_context(tc.tile_pool(name="vpool", bufs=2))
        epool = ctx.enter_context(tc.tile_pool(name="epool", bufs=3))
        opool = ctx.enter_context(tc.tile_pool(name="opool", bufs=4))
        psS = ctx.enter_context(tc.tile_pool(name="psS", bufs=2, space="PSUM"))
        psO = ctx.enter_context(tc.tile_pool(name="psO", bufs=2, space="PSUM"))

        # ---- persistent tiles ----
        w3_sb = persist.tile([128, 3 * KC * 128], BF16, tag="w3")
        wo_sb = persist.tile([128, 8 * E], BF16, tag="wo")
        cf = persist.tile([128, 12], F32, tag="cf")
        cb = persist.tile([128, 272], BF16, tag="cb")
        ind_sb = persist.tile([2, 128], BF16, tag="ind")

        QT = persist.tile([128, L], BF16, tag="QT")
        KT = persist.tile([128, L], BF16, tag="KT")
        VT = persist.tile([128, L], BF16, tag="VT")
        QT2 = persist.tile([128, G], BF16, tag="QT2")
        KT2 = persist.tile([128, G], BF16, tag="KT2")
        VT2 = persist.tile([128, G], BF16, tag="VT2")
        acc = persist.tile([128, L], F32, tag="acc")
        zz = persist.tile([33, L], F32, tag="zz")
        accb = persist.tile([128, L], BF16, tag="accb")
        mgr0 = persist.tile([128, 8 * CW], BF16, tag="mgr")
        mgr1 = persist.tile([128, 8 * CW], BF16, tag="mgr")
        mgr = [mgr0, mgr1]
        zw = persist.tile([128, 64], F32, tag="zw")
        zwb = persist.tile([128, 64], BF16, tag="zwb")
        rzp = persist.tile([2, L], BF16, tag="rzp")

        TRI = cb[:, 0:128]
        ONES16 = cb[:, 256:272]

        def eye_s(slot):
            return cb[slot * 64:(slot + 1) * 64,
                      128 + slot * 64:128 + (slot + 1) * 64]

        # ---- weights first (split per stream; 2KB lines), then consts ----
        for soff, eng in ((0, nc.sync), (1, nc.gpsimd), (2, nc.scalar)):
            eng.dma_start(w3_sb[:, soff * 1024:(soff + 1) * 1024],
                          w3[:, soff * 1024:(soff + 1) * 1024])
        nc.gpsimd.dma_start(cf[:], constf[:])
        nc.gpsimd.dma_start(cb[:], constb[:])
        nc.gpsimd.dma_start(ind_sb[:], ind2d[:])
        for i in range(4):
            nc.scalar.dma_start(wo_sb[:, i * 2048:(i + 1) * 2048],
                                wo[:, i * 2048:(i + 1) * 2048])

        # warm the ACT exp table early
        wtile = opool.tile([1, 16], BF16, tag="warm")
        nc.scalar.activation(wtile[:], ONES16[0:1, 0:16], AF.Exp)

        # tiny dummy collective absorbs first-collective setup cost;
        # emitted before the xin trigger flood so the gpsimd queue reaches it
        for rr in range(8):
            nc.scalar.dma_start(a2a_warm_in[rr][0:1, 0:16], ONES16[0:1, 0:16])
        nc.gpsimd.collective_compute(
            "AllToAll", mybir.AluOpType.bypass,
            replica_groups=[list(range(8))],
            ins=[a2a_warm_in[:]], outs=[a2a_warm_out[:]],
        )

        # ---- stream all input position blocks up-front ----
        streams = (("k", kT, KT, 0), ("v", vT, VT, 1), ("q", qT, QT, 2))
        xin_tiles = {}
        qengs = (nc.sync, nc.gpsimd)
        qi = 0
        for pb in range(NPP):
            for name, x_d, _, _ in streams:
                xr = x_d.rearrange("(kc p) l -> kc p l", p=128)
                xin = pin.tile([128, KC * PBP], BF16, tag="xin")
                for kc in range(KC):
                    qengs[qi % 2].dma_start(
                        xin[:, kc * PBP:(kc + 1) * PBP],
                        xr[kc][:, pb * PBP:(pb + 1) * PBP],
                    )
                    qi += 1
                xin_tiles[(pb, name)] = xin

        # ---- projection work quanta ----
        def proj_quantum(pb, sname, half):
            xin = xin_tiles[(pb, sname)]
            _, _, dst, soff = next(s for s in streams if s[0] == sname)
            pt = psS.tile([128, 512], F32, tag="ps")
            c0 = half * 512
            for kc in range(KC):
                nc.tensor.matmul(
                    pt[:, 0:512],
                    w3_sb[:, soff * 1024 + kc * 128:soff * 1024 + (kc + 1) * 128],
                    xin[:, kc * PBP + c0:kc * PBP + c0 + 512],
                    start=(kc == 0), stop=(kc == KC - 1),
                )
            dslice = dst[:, pb * PBP + c0:pb * PBP + c0 + 512]
            if sname == "q":
                nc.vector.tensor_scalar_add(dslice, pt[:, 0:512], cf[:, 0:1])
            elif sname == "k":
                nc.vector.tensor_scalar_add(dslice, pt[:, 0:512], cf[:, 1:2])
            else:
                nc.vector.tensor_copy(dslice, pt[:, 0:512])

        def quanta_for_pb(pb):
            return [(lambda p=pb, s=s, h=h: proj_quantum(p, s, h))
                    for s in ("k", "v", "q") for h in (0, 1)]

        # ---- branch-2 dilation-compressed copies (per pb chunk) ----
        def b2_part(pb):
            for src, dst in ((QT, QT2), (KT, KT2), (VT, VT2)):
                for slot in range(2):
                    p0 = 64 * slot
                    o0 = 2 * slot
                    dc = dst[p0:p0 + 64, pb * 256:(pb + 1) * 256]
                    s0 = pb * PBP + o0
                    nc.vector.tensor_scalar_mul(
                        dc,
                        src[p0:p0 + 64, s0:s0 + 4 * 255 + 1:4],
                        cf[p0:p0 + 64, 10:11],
                    )
                    nc.vector.scalar_tensor_tensor(
                        dc,
                        src[p0:p0 + 64, s0 + 1:s0 + 1 + 4 * 255 + 1:4],
                        cf[p0:p0 + 64, 11:12], dc,
                        mybir.AluOpType.mult, mybir.AluOpType.add,
                    )

        # ---- normalization helpers (per position-half) ----
        def norm_gather(h):
            for slot in range(2):
                blk = 2 * slot + h
                nc.sync.dma_start(
                    zw[:, blk * 16:blk * 16 + 16],
                    zz[32 * slot:32 * slot + 1, 2048 * h:2048 * (h + 1)],
                )
            zw4 = zw[:].rearrange("p (s b c) -> p s b c", s=2, b=2)
            zwb4 = zwb[:].rearrange("p (s b c) -> p s b c", s=2, b=2)
            with nc.allow_low_precision(reason="softmax denom reciprocal"):
                nc.vector.reciprocal(zw4[:, :, h, :], zw4[:, :, h, :])
            nc.vector.tensor_copy(zwb4[:, :, h, :], zw4[:, :, h, :])
            for slot in range(2):
                blk = 2 * slot + h
                nc.sync.dma_start(
                    rzp[slot:slot + 1, 2048 * h:2048 * (h + 1)],
                    zwb[:, blk * 16:blk * 16 + 16],
                )

        def norm_block(pb8):
            rb = psS.tile([128, 512], F32, tag="ps")
            nc.tensor.matmul(rb[:, 0:512], ind_sb[:],
                             rzp[:, pb8 * PB:(pb8 + 1) * PB],
                             start=True, stop=True)
            with nc.allow_low_precision(reason="bf16 a2a payload"):
                nc.vector.tensor_mul(
                    accb[:, pb8 * PB:(pb8 + 1) * PB],
                    acc[:, pb8 * PB:(pb8 + 1) * PB], rb[:, 0:512])

        # ---- K/Q slicing per branch ----
        def kq_slice(br, seg, slot, t, lo, size):
            if br == 0:
                base = 1024 * seg + lo
                return t[slot * 64:(slot + 1) * 64, base:base + size]
            if br == 1:
                base = 2048 * seg + 2 * lo + slot
                return t[slot * 64:(slot + 1) * 64,
                         base:base + 2 * size - slot:2]
            return t[slot * 64:(slot + 1) * 64, lo:lo + size]

        # ---- one (branch, segment) job ----
        def job(br, seg, fillers):
            kt_src = KT2 if br == 2 else KT
            qt_src = QT2 if br == 2 else QT
            fillers = list(fillers)

            # V_both prep: PE transposes + DVE copies into vb
            vb = vpool.tile([128, 2 * 8 * VBW], BF16, tag="vb")
            nc.vector.tensor_copy(vb[:, 64::VBW], ONES16)
            for jc in range(8):
                if br == 0:
                    tp = psS.tile([128, 128], BF16, tag="ps")
                    src = VT[:, 1024 * seg + 128 * jc:1024 * seg + 128 * (jc + 1)]
                    nc.tensor.transpose(tp[:, 0:128], src, cb[:, 128:256])
                    dst = vb[:].rearrange(
                        "p (s jj t) -> p s jj t", s=2, jj=8
                    )[:, :, jc, 0:64]
                    srcp = tp[:, 0:128].rearrange("p (s r) -> p s r", s=2)
                    nc.vector.tensor_copy(dst, srcp)
                else:
                    for slot in range(2):
                        tp = psS.tile([128, 128], BF16, tag="ps")
                        if br == 1:
                            base = 2048 * seg + 256 * jc + slot
                            src = VT[slot * 64:(slot + 1) * 64,
                                     base:base + 256 - slot:2]
                        else:
                            src = VT2[slot * 64:(slot + 1) * 64,
                                      128 * jc:128 * (jc + 1)]
                        nc.tensor.transpose(tp[:, 0:64], src, eye_s(slot))
                        nc.vector.tensor_copy(
                            vb[:, slot * 8 * VBW + jc * VBW:
                               slot * 8 * VBW + jc * VBW + 64],
                            tp[:, 0:64],
                        )

            o_ps_a = psO.tile([128, 1024], F32, tag="o")
            o_ps_b = psO.tile([128, 1024], F32, tag="o")
            o_ps = [o_ps_a, o_ps_b]

            def merge(r0, r1):
                # scatter o_ps[slot] region [r0:r1] (+ Z row) into acc/zz
                for slot in range(2):
                    op = o_ps[slot]
                    po = slot * 64
                    zr = 32 * slot
                    w = r1 - r0
                    if br == 0:
                        d0 = 1024 * seg + r0
                        if slot == 0:
                            nc.vector.tensor_copy(
                                acc[po:po + 64, d0:d0 + w], op[0:64, r0:r1])
                            nc.vector.tensor_copy(
                                zz[zr:zr + 1, d0:d0 + w], op[64:65, r0:r1])
                        else:
                            nc.scalar.copy(
                                acc[po:po + 64, d0:d0 + w], op[0:64, r0:r1])
                            nc.scalar.copy(
                                zz[zr:zr + 1, d0:d0 + w], op[64:65, r0:r1])
                    elif br == 1:
                        d0 = 2048 * seg + 2 * r0 + slot
                        d1 = d0 + 2 * (w - 1) + 1
                        ac = acc[po:po + 64, d0:d1:2]
                        nc.vector.tensor_add(ac, ac, op[0:64, r0:r1])
                        zc = zz[zr:zr + 1, d0:d1:2]
                        nc.vector.tensor_add(zc, zc, op[64:65, r0:r1])
                    else:
                        o0 = 2 * slot
                        for dd in range(2):
                            d0 = 4 * r0 + o0 + dd
                            d1 = d0 + 4 * (w - 1) + 1
                            ac = acc[po:po + 64, d0:d1:4]
                            nc.vector.scalar_tensor_tensor(
                                ac, op[0:64, r0:r1],
                                cf[po:po + 64, 10 + dd:11 + dd],
                                ac, mybir.AluOpType.mult, mybir.AluOpType.add,
                            )
                            zc = zz[zr:zr + 1, d0:d1:4]
                            nc.vector.scalar_tensor_tensor(
                                zc, op[64:65, r0:r1],
                                cf[zr:zr + 1, 10 + dd:11 + dd],
                                zc, mybir.AluOpType.mult, mybir.AluOpType.add,
                            )

            for jc in range(8):
                c0 = 128 * jc
                es = []
                for slot in range(2):
                    s = psS.tile([128, 1024], F32, tag="ps")
                    lhs = kq_slice(br, seg, slot, kt_src, c0, 128)
                    # causal-mask bias first: clears the bank (start=True),
                    # writes -100 upper-triangle into the diagonal block.
                    nc.tensor.matmul(
                        s[:, c0:c0 + 128], cb[:, 128:256], TRI,
                        start=True, stop=False,
                    )
                    if c0 < 512:
                        nc.tensor.matmul(
                            s[:, c0:512], lhs,
                            kq_slice(br, seg, slot, qt_src, c0, 512 - c0),
                            start=False, stop=True,
                            tile_position=(slot * 64, 0),
                        )
                        nc.tensor.matmul(
                            s[:, 512:1024], lhs,
                            kq_slice(br, seg, slot, qt_src, 512, 512),
                            start=True, stop=True,
                            tile_position=(slot * 64, 0),
                        )
                    else:
                        nc.tensor.matmul(
                            s[:, c0:1024], lhs,
                            kq_slice(br, seg, slot, qt_src, c0, 1024 - c0),
                            start=False, stop=True,
                            tile_position=(slot * 64, 0),
                        )
                    e = epool.tile([128, 1024], BF16, tag="e")
                    nc.scalar.activation(e[:, c0:1024], s[:, c0:1024], AF.Exp)
                    es.append(e)

                if fillers:
                    f = fillers.pop(0)
                    if f is not None:
                        f()

                for slot in range(2):
                    e = es[slot]
                    vbs = vb[:, slot * 8 * VBW + jc * VBW:
                             slot * 8 * VBW + (jc + 1) * VBW]
                    if c0 < 512:
                        nc.tensor.matmul(
                            o_ps[slot][0:VBW, c0:512], vbs, e[:, c0:512],
                            start=(jc == 0), stop=(jc == 3),
                        )
                        nc.tensor.matmul(
                            o_ps[slot][0:VBW, 512:1024], vbs, e[:, 512:1024],
                            start=(jc == 0), stop=(jc == 7),
                        )
                    else:
                        nc.tensor.matmul(
                            o_ps[slot][0:VBW, c0:1024], vbs, e[:, c0:1024],
                            start=(jc == 0), stop=(jc == 7),
                        )
                if jc == 3:
                    merge(0, 512)
            merge(512, 1024)
            for f in fillers:
                if f is not None:
                    f()

        # ================= emission order =================
        for q in quanta_for_pb(0):
            q()
        b2_part(0)
        job(0, 0, quanta_for_pb(1))
        b2_part(1)
        job(0, 1, quanta_for_pb(2))
        b2_part(2)
        job(1, 0, quanta_for_pb(3))
        b2_part(3)
        job(0, 2, [])
        job(0, 3, [])
        job(2, 0, [])
        # half-0 normalization rides inside the last job as filler
        nh0 = [lambda: norm_gather(0)] + \
              [(lambda p=p: norm_block(p)) for p in range(4)]
        job(1, 1, nh0)

        # ---- half-1 normalization ----
        norm_gather(1)
        for p in range(4, 8):
            norm_block(p)

        # ---- chunked AllToAll + output projection pipeline ----
        # batched staging: one trigger per chunk instead of 16
        for h in range(2):
            nc.sync.dma_start(
                a2a_in[h][:].rearrange("r p c -> p r c"),
                accb[:].rearrange("p (r hh c) -> p r hh c", r=8, hh=2)[:, :, h, :],
            )
        for h in range(2):
            nc.gpsimd.collective_compute(
                "AllToAll", mybir.AluOpType.bypass,
                replica_groups=[list(range(8))],
                ins=[a2a_in[h][:]], outs=[a2a_out[h][:]],
            )
        for h in range(2):
            nc.sync.dma_start(
                mgr[h][:].rearrange("p (s c) -> p s c", s=8),
                a2a_out[h][:].rearrange("s p c -> p s c"),
            )
        for h in range(2):
            for ob in range(8):
                pt = psS.tile([128, 512], F32, tag="ps")
                for ec in range(KC):
                    nc.tensor.matmul(
                        pt[:, 0:CW],
                        wo_sb[:, ec * E + ob * 128:ec * E + (ob + 1) * 128],
                        mgr[h][:, ec * CW:(ec + 1) * CW],
                        start=(ec == 0), stop=(ec == KC - 1),
                    )
                osb = opool.tile([128, CW], BF16, tag="osb")
                nc.vector.tensor_scalar_add(osb[:], pt[:, 0:CW],
                                            cf[:, 2 + ob:3 + ob])
                nc.gpsimd.dma_start(
                    outT[ob * 128:(ob + 1) * 128, h * CW:(h + 1) * CW],
                    osb[:])

    with tile.TileContext(nc) as tc, contextlib.ExitStack() as ctx:
        _emit(tc, ctx)

    nc.compile()
    return nc


_NC_CACHE = {}


def _get_nc():
    if "nc" not in _NC_CACHE:
        _NC_CACHE["nc"] = _build()
    return _NC_CACHE["nc"]


def _prep_inputs(query, key, value, Wq, bq, Wk, bk, Wv, bv, Wo, bo):
    """Host-side layout prep. Returns in_maps for the 8 cores."""
    qT = np.ascontiguousarray(query[0].T).astype(BF)   # (E, L)
    kT = np.ascontiguousarray(key[0].T).astype(BF)
    vT = np.ascontiguousarray(value[0].T).astype(BF)

    WqT = np.ascontiguousarray(Wq.T) * np.float32(0.125)
    WkT = np.ascontiguousarray(Wk.T)
    WvT = np.ascontiguousarray(Wv.T)

    # permuted Wo.T rows to match a2a arriving-feature order
    perm = np.concatenate(
        [np.r_[64 * s:64 * s + 64, 512 + 64 * s:512 + 64 * s + 64]
         for s in range(8)]
    )
    WoT = np.ascontiguousarray(Wo.T)[perm]            # (E e', E o)
    wo_pack = np.zeros((128, 8 * E), np.float32)
    for ec in range(8):
        wo_pack[:, ec * E:(ec + 1) * E] = WoT[ec * 128:(ec + 1) * 128]
    wo_pack = wo_pack.astype(BF)

    bo_eff = (bo + bv @ Wo.T).astype(np.float32)
    bo8 = bo_eff.reshape(8, 128).T.copy()             # [p, ob]

    IND = np.zeros((2, 128), np.float32)
    IND[0, 0:64] = 1.0
    IND[1, 64:128] = 1.0
    # [k, q] layout: mask where q < k
    TRI = np.zeros((128, 128), np.float32)
    ki, qi = np.meshgrid(np.arange(128), np.arange(128), indexing="ij")
    TRI[qi < ki] = -100.0
    EYE = np.eye(128, dtype=np.float32)

    constb = np.zeros((128, 272), np.float32)
    constb[:, 0:128] = TRI
    constb[:, 128:256] = EYE
    constb[:, 256:272] = 1.0
    constb = constb.astype(BF)

    in_maps = []
    for c in range(8):
        fa = np.r_[64 * c:64 * c + 64]
        fb = np.r_[512 + 64 * c:512 + 64 * c + 64]
        sel = np.concatenate([fa, fb])
        w3 = np.zeros((128, 3 * KC * 128), np.float32)
        for soff, WT in ((0, WkT), (1, WvT), (2, WqT)):
            for kc in range(KC):
                w3[:, soff * 1024 + kc * 128:soff * 1024 + (kc + 1) * 128] = \
                    WT[kc * 128:(kc + 1) * 128][:, sel]
        constf = np.zeros((128, 12), np.float32)
        constf[:, 0] = bq[sel] * np.float32(0.125)
        constf[:, 1] = bk[sel]
        constf[:, 2:10] = bo8
        constf[:, 10 + c // 4] = 1.0   # branch-2 offset indicator
        in_maps.append({
            "qT": qT, "kT": kT, "vT": vT,
            "w3": w3.astype(BF),
            "wo": wo_pack,
            "constf": constf,
            "constb": constb,
            "ind2d": IND.astype(BF),
        })
    return in_maps


def kernel(query, key, value, Wq, bq, Wk, bk, Wv, bv, Wo, bo,
           _trace=False, _result_holder=None):
    args = [np.asarray(a, np.float32) for a in
            (query, key, value, Wq, bq, Wk, bk, Wv, bv, Wo, bo)]
    nc = _get_nc()
    in_maps = _prep_inputs(*args)
    res = bass_utils.run_bass_kernel_spmd(
        nc, in_maps, core_ids=list(range(N_CORES)), trace=_trace
    )
    if _result_holder is not None:
        _result_holder.append(res)
    outT = np.zeros((E, L), np.float32)
    for c in range(N_CORES):
        outT[:, PB * c:PB * (c + 1)] = res.results[c]["outT"]
    return np.ascontiguousarray(outT.T).reshape(1, L, E)
